# revision 10
# baseline (speedup 1.0000x reference)
"""GAT with autoencoder + residuals on 8 trn2 NeuronCores (Bass/Tile).

Strategy: nodes are renumbered by in-degree (desc) and dealt round-robin to the
8 cores; each dst-node owns one SBUF partition of its 128-node tile, and its
incoming edges occupy chunk columns of that partition. Gathers of source
features use dma_gather (int16 indices -> lo/hi split tables), split across
the 4 SWDGE queues to avoid ring backpressure. Attention weights are
normalized (alpha = e/sum e) BEFORE aggregation; aggregation runs
data-stationary on the tensor engine (lhsT = gathered chunk, rhs = identity),
accumulating feature-major directly on top of the residual-projection PSUM.
All dense math in bf16. Small weights are replicated; two AllGathers publish
the per-core projection tables between phases.
"""
import hashlib
import numpy as np
import ml_dtypes

import concourse.bacc as bacc
import concourse.mybir as mybir
import concourse.tile as tile
from concourse import bass_utils

# model sizes (fixed by the problem)
N = 50000
IN = 512
ENC = 256
HID = 32
HEADS = 4
OUT = 40
SLOPE = 0.2

NCORE = 8
P = 128
NTILE = 49
NPC = NTILE * P          # 6272 nodes per core
NPAD = NCORE * NPC       # 50176
LO = 32768               # lo table rows [0, LO)
HIOFF = NPAD - 32768     # hi table rows [HIOFF, NPAD)
# chunk-major table layout: tables are built in CHUNKS of tile ranges so the
# AllGather of chunk k can launch as soon as its producer groups finish.
CH_T = [0, 16, 28, 40, 49]           # tile boundaries of the 4 chunks
CH_R0 = [t * P for t in CH_T]        # per-core row boundaries
CH_CR = [CH_R0[i + 1] - CH_R0[i] for i in range(4)]   # rows per core per chunk
CH_CB = [0]
for i in range(4):
    CH_CB.append(CH_CB[-1] + NCORE * CH_CR[i])        # table base of chunk

F32 = mybir.dt.float32
BF16 = mybir.dt.bfloat16
I16 = mybir.dt.int16
AF = mybir.ActivationFunctionType
ALU = mybir.AluOpType
AX = mybir.AxisListType

TB1 = 256   # bf16 columns per table1 row: [h(128 bf16) | a_src(4 f32) | pad]
TB2 = 128   # bf16 columns per table2 row: [z(40 bf16) | b_src(1 bf16) | pad]
SUBCH = 6   # max chunks per sub-gather

_cache = {}


def _wrap_idx(blk):
    """[128, NB] slot-major block -> dma_gather idx layout [128, 8*NB] int16."""
    nb = blk.shape[1]
    flat = blk.T.reshape(-1)                 # j = c*128 + p
    w = flat.reshape(-1, 16).T               # [16, 8*NB]
    return np.tile(w, (8, 1)).astype(np.int16)


def _prepare(inputs):
    x = np.asarray(inputs["x"], np.float32)
    ei = np.asarray(inputs["edge_index"]).astype(np.int64)
    src = np.concatenate([ei[0], np.arange(N, dtype=np.int64)])
    dst = np.concatenate([ei[1], np.arange(N, dtype=np.int64)])

    deg = np.bincount(dst, minlength=NPAD)
    order = np.argsort(-deg, kind="stable")
    rank = np.empty(NPAD, np.int64)
    rank[order] = np.arange(NPAD)
    core_of = rank % NCORE
    pos_of = rank // NCORE
    tidx_of = core_of * NPC + pos_of         # table row of each (old) node

    er = rank[dst]
    est = tidx_of[src]
    lo_ex = est < HIOFF
    hi_ex = est >= LO
    key2 = np.where(lo_ex, 0, np.where(hi_ex, 2, 1))

    nlo = np.bincount(er[lo_ex], minlength=NPAD)
    nhi = np.bincount(er[hi_ex], minlength=NPAD)
    degr = deg[order]

    NLO = np.zeros(NTILE, np.int64)
    NHI = np.zeros(NTILE, np.int64)
    l_of = np.zeros(NPAD, np.int64)
    B = NCORE * P
    for t in range(NTILE):
        blk = slice(t * B, (t + 1) * B)
        NLO[t] = nlo[blk].max()
        l = np.minimum(degr[blk] - nhi[blk], NLO[t])
        l_of[blk] = l
        NHI[t] = max(nhi[blk].max(), (degr[blk] - l).max(), 0)
    NC = NLO + NHI
    CSTART = np.zeros(NTILE, np.int64)
    CSTART[1:] = np.cumsum(NC)[:-1]
    CTOT = int(NC.sum())

    # per-edge slot assignment
    eord = np.lexsort((key2, er))
    er_s = er[eord]
    est_s = est[eord]
    boundaries = np.flatnonzero(np.r_[True, er_s[1:] != er_s[:-1]])
    counts = np.diff(np.r_[boundaries, er_s.size])
    j = np.arange(er_s.size) - np.repeat(boundaries, counts)
    lcap = l_of[er_s]
    side_lo = j < lcap
    et_s = (er_s // NCORE) // P
    epart_s = (er_s // NCORE) % P
    ecore_s = er_s % NCORE
    col = np.where(side_lo, j, NLO[et_s] + (j - lcap))
    val = np.where(side_lo, est_s, est_s - HIOFF).astype(np.int16)

    sval = np.zeros((NCORE, P, CTOT), np.int16)
    mask = np.zeros((NCORE, P, CTOT), np.float32)
    colg = CSTART[et_s] + col
    sval[ecore_s, epart_s, colg] = val
    mask[ecore_s, epart_s, colg] = 1.0

    # wrapped idx blocks per (tile, side) concatenated; WSTART per gather
    WSTART = []
    w_off = 0
    for t in range(NTILE):
        WSTART.append((w_off, w_off + 8 * int(NLO[t])))
        w_off += 8 * int(NLO[t]) + 8 * int(NHI[t])
    WTOT = w_off
    idx_all = np.zeros((NCORE, P, WTOT), np.int16)
    for c in range(NCORE):
        for t in range(NTILE):
            cs = CSTART[t]
            lo_w, hi_w = WSTART[t]
            if NLO[t]:
                idx_all[c][:, lo_w:lo_w + 8 * int(NLO[t])] = _wrap_idx(
                    sval[c][:, cs:cs + int(NLO[t])])
            if NHI[t]:
                idx_all[c][:, hi_w:hi_w + 8 * int(NHI[t])] = _wrap_idx(
                    sval[c][:, cs + int(NLO[t]):cs + int(NC[t])])

    # per-core x (transposed, feature-major, bf16)
    xp = np.zeros((NPAD, IN), np.float32)
    xp[:N] = x
    old_ids = np.empty((NCORE, NPC), np.int64)
    x_t = np.empty((NCORE, IN, NPC), ml_dtypes.bfloat16)
    for c in range(NCORE):
        ids = order[c + NCORE * np.arange(NPC)]
        old_ids[c] = ids
        x_t[c] = xp[ids].T.astype(ml_dtypes.bfloat16)

    # replicated derived weights
    w = lambda k: np.asarray(inputs[k], np.float32)
    bf = lambda a: np.ascontiguousarray(a).astype(ml_dtypes.bfloat16)
    a1sd = np.zeros((P, 8), np.float32)
    for h in range(HEADS):
        a1sd[h * HID:(h + 1) * HID, h] = w("gat1_att_src")[h]
        a1sd[h * HID:(h + 1) * HID, 4 + h] = w("gat1_att_dst")[h]
    vs2 = w("gat2_w") @ w("gat2_att_src")[0]
    vd2 = w("gat2_w") @ w("gat2_att_dst")[0]
    lhsT2 = np.concatenate([w("gat2_w"), vs2[:, None], vd2[:, None]], 1)  # [128,42]
    bias12 = (w("gat1_b") + w("res1_b")).reshape(P, 1)
    b2c = np.zeros((P, 1), np.float32)
    b2c[:OUT, 0] = w("gat2_b") + w("res2_b")

    consts = {
        "identb": np.eye(P, dtype=np.float32).astype(ml_dtypes.bfloat16),
        "ident": np.eye(P, dtype=np.float32),
        "w1": bf(w("ae_w1")), "b1p": w("ae_b1").reshape(2, P).T.copy(),
        "w2": bf(w("ae_w2")), "b2p": w("ae_b2").reshape(2, P).T.copy(),
        "g1w": bf(w("gat1_w")), "a1sd": bf(a1sd),
        "res1w": bf(w("res1_w")), "b12p": bias12,
        "l2p": bf(lhsT2), "r2w": bf(w("res2_w")), "b2cp": b2c,
    }
    meta = {
        "NLO": NLO.tolist(), "NHI": NHI.tolist(),
        "CSTART": CSTART.tolist(), "CTOT": CTOT,
        "WSTART": WSTART, "WTOT": WTOT,
        "key": "v2:" + hashlib.sha1(ei.tobytes()).hexdigest(),
    }
    in_maps = []
    for c in range(NCORE):
        m = {"x_t": x_t[c], "idx_all": idx_all[c], "mask_all": mask[c]}
        m.update(consts)
        in_maps.append(m)
    return meta, in_maps, old_ids


def _build(meta):
    NLO, NHI = meta["NLO"], meta["NHI"]
    CSTART, CTOT = meta["CSTART"], meta["CTOT"]
    WSTART, WTOT = meta["WSTART"], meta["WTOT"]

    nc = bacc.Bacc("TRN2", target_bir_lowering=False, debug=False,
                   num_devices=NCORE, num_swdge_queues=4)
    # I/O
    x_t = nc.dram_tensor("x_t", [IN, NPC], BF16, kind="ExternalInput")
    idx_all = nc.dram_tensor("idx_all", [P, WTOT], I16, kind="ExternalInput")
    mask_all = nc.dram_tensor("mask_all", [P, CTOT], F32, kind="ExternalInput")
    identb = nc.dram_tensor("identb", [P, P], BF16, kind="ExternalInput")
    ident = nc.dram_tensor("ident", [P, P], F32, kind="ExternalInput")
    w1 = nc.dram_tensor("w1", [IN, ENC], BF16, kind="ExternalInput")
    b1p = nc.dram_tensor("b1p", [P, 2], F32, kind="ExternalInput")
    w2 = nc.dram_tensor("w2", [ENC, ENC], BF16, kind="ExternalInput")
    b2p = nc.dram_tensor("b2p", [P, 2], F32, kind="ExternalInput")
    g1w = nc.dram_tensor("g1w", [ENC, P], BF16, kind="ExternalInput")
    a1sd = nc.dram_tensor("a1sd", [P, 8], BF16, kind="ExternalInput")
    res1w = nc.dram_tensor("res1w", [ENC, P], BF16, kind="ExternalInput")
    b12p = nc.dram_tensor("b12p", [P, 1], F32, kind="ExternalInput")
    l2p = nc.dram_tensor("l2p", [P, 42], BF16, kind="ExternalInput")
    r2w = nc.dram_tensor("r2w", [P, OUT], BF16, kind="ExternalInput")
    b2cp = nc.dram_tensor("b2cp", [P, 1], F32, kind="ExternalInput")
    out_d = nc.dram_tensor("out", [OUT, NPC], F32, kind="ExternalOutput")

    groups = [(gi, min(4, NTILE - gi * 4)) for gi in range((NTILE + 3) // 4)]

    with tile.TileContext(nc) as tc:
        with (
            tc.tile_pool(name="const", bufs=1) as cp,
            tc.tile_pool(name="pers", bufs=1) as pp,
            tc.tile_pool(name="dram", bufs=1, space="DRAM") as dp,
            tc.tile_pool(name="xk", bufs=2) as xkp,
            tc.tile_pool(name="wk", bufs=2) as wk,
            tc.tile_pool(name="tb", bufs=3) as tbp,
            tc.tile_pool(name="gg", bufs=3) as gp,
            tc.tile_pool(name="gg2", bufs=2) as gp2,
            tc.tile_pool(name="pbig", bufs=2, space="PSUM") as pb,
            tc.tile_pool(name="pzo", bufs=2, space="PSUM") as pz,
            tc.tile_pool(name="ptr", bufs=4, space="PSUM") as ptp,
        ):
            # ---- load constants
            identb_sb = cp.tile([P, P], BF16)
            nc.sync.dma_start(identb_sb[:], identb[:])
            ident_sb = cp.tile([P, P], F32)
            nc.sync.dma_start(ident_sb[:], ident[:])
            w1_sb = cp.tile([P, 4 * ENC], BF16)
            nc.sync.dma_start(
                w1_sb[:].rearrange("p (k m) -> p k m", m=ENC),
                w1[:].rearrange("(k p) m -> p k m", p=P))
            w2_sb = cp.tile([P, 2 * ENC], BF16)
            nc.sync.dma_start(
                w2_sb[:].rearrange("p (k m) -> p k m", m=ENC),
                w2[:].rearrange("(k p) m -> p k m", p=P))
            g1w_sb = cp.tile([P, 2 * P], BF16)
            nc.sync.dma_start(
                g1w_sb[:].rearrange("p (k m) -> p k m", m=P),
                g1w[:].rearrange("(k p) m -> p k m", p=P))
            res1w_sb = cp.tile([P, 2 * P], BF16)
            nc.sync.dma_start(
                res1w_sb[:].rearrange("p (k m) -> p k m", m=P),
                res1w[:].rearrange("(k p) m -> p k m", p=P))
            b1_sb = cp.tile([P, 2], F32)
            nc.sync.dma_start(b1_sb[:], b1p[:])
            b2_sb = cp.tile([P, 2], F32)
            nc.sync.dma_start(b2_sb[:], b2p[:])
            a1sd_sb = cp.tile([P, 8], BF16)
            nc.sync.dma_start(a1sd_sb[:], a1sd[:])
            b12_sb = cp.tile([P, 1], F32)
            nc.sync.dma_start(b12_sb[:], b12p[:])
            l2_sb = cp.tile([P, 42], BF16)
            nc.sync.dma_start(l2_sb[:], l2p[:])
            r2w_sb = cp.tile([P, OUT], BF16)
            nc.sync.dma_start(r2w_sb[:], r2w[:])
            b2c_sb = cp.tile([P, 1], F32)
            nc.sync.dma_start(b2c_sb[:], b2cp[:])
            idx_sb = pp.tile([P, WTOT], I16)
            nc.sync.dma_start(idx_sb[:], idx_all[:])
            mask_sb = pp.tile([P, CTOT], F32)
            nc.sync.dma_start(mask_sb[:], mask_all[:])

            adst_nm = pp.tile([P, NTILE * 4], F32)
            bdst_nm = pp.tile([P, NTILE], F32)
            xe_res = pp.tile([P, 2 * NPC], BF16)       # encoder output, resident
            res2F = pp.tile([OUT, NPC], F32)           # res2 branch, feature-major

            # internal DRAM
            slice1 = dp.tile([NPC, TB1], BF16)
            full1 = dp.tile([NPAD, TB1], BF16, addr_space="Shared")
            slice2 = dp.tile([NPC, TB2], BF16)
            full2 = dp.tile([NPAD, TB2], BF16, addr_space="Shared")

            # ---- phase A+B: autoencoder, GAT1 projections, table1 rows
            for gi, gn in groups:
                GW = gn * P
                g0 = gi * 4 * P
                xks = []
                for k in range(4):
                    xk = xkp.tile([P, GW], BF16, tag=f"xk{k}")
                    nc.sync.dma_start(xk[:], x_t[k * P:(k + 1) * P, g0:g0 + GW])
                    xks.append(xk)
                z1s = []
                for m in range(2):
                    ps1 = pb.tile([P, GW], F32, tag="pbig")
                    for k in range(4):
                        nc.tensor.matmul(
                            out=ps1[:], lhsT=w1_sb[:, k * ENC + m * P:k * ENC + (m + 1) * P],
                            rhs=xks[k][:], start=(k == 0), stop=(k == 3))
                    z1 = wk.tile([P, GW], BF16, tag=f"z1{m}")
                    nc.scalar.activation(z1[:], ps1[:], AF.Relu, bias=b1_sb[:, m:m + 1])
                    z1s.append(z1)
                for m in range(2):
                    ps2 = pb.tile([P, GW], F32, tag="pbig")
                    for k in range(2):
                        nc.tensor.matmul(
                            out=ps2[:], lhsT=w2_sb[:, k * ENC + m * P:k * ENC + (m + 1) * P],
                            rhs=z1s[k][:], start=(k == 0), stop=(k == 1))
                    nc.scalar.activation(xe_res[:, m * NPC + g0:m * NPC + g0 + GW],
                                         ps2[:], AF.Relu, bias=b2_sb[:, m:m + 1])
                # h = xe @ gat1_w ; a_src/a_dst
                psh = pb.tile([P, GW], F32, tag="pbig")
                for k in range(2):
                    nc.tensor.matmul(out=psh[:], lhsT=g1w_sb[:, k * P:(k + 1) * P],
                                     rhs=xe_res[:, k * NPC + g0:k * NPC + g0 + GW],
                                     start=(k == 0), stop=(k == 1))
                h_sb = wk.tile([P, GW], BF16, tag="hsb")
                nc.vector.tensor_copy(h_sb[:], psh[:])
                psa = pz.tile([8, GW], F32, tag="pzo")
                nc.tensor.matmul(out=psa[:], lhsT=a1sd_sb[:], rhs=h_sb[:],
                                 start=True, stop=True)
                a_sd = wk.tile([8, GW], F32, tag="asd")
                nc.vector.tensor_copy(a_sd[:], psa[:])
                for s in range(gn):
                    ti = gi * 4 + s
                    ptr1 = ptp.tile([P, P], BF16, tag="ptr")
                    nc.tensor.transpose(ptr1[:], h_sb[:, s * P:(s + 1) * P], identb_sb[:])
                    tb1 = tbp.tile([P, TB1], BF16, tag="tb1")
                    nc.vector.tensor_copy(tb1[:, 0:P], ptr1[:])
                    ptr2 = ptp.tile([P, 8], F32, tag="ptr")
                    nc.tensor.transpose(ptr2[:], a_sd[:, s * P:(s + 1) * P], ident_sb[0:8, 0:8])
                    f32v = tb1[:].bitcast(F32)
                    nc.vector.tensor_copy(f32v[:, 64:68], ptr2[:, 0:4])
                    nc.vector.tensor_copy(adst_nm[:, ti * 4:(ti + 1) * 4], ptr2[:, 4:8])
                    nc.sync.dma_start(slice1[ti * P:(ti + 1) * P, :], tb1[:])

            # ---- AllGather table1
            nc.gpsimd.collective_compute(
                "AllGather", ALU.bypass, replica_groups=[list(range(NCORE))],
                ins=[slice1[:]], outs=[full1[:]])

            qrot = [0]

            def split_gather(G, ti, tb, full):
                """4-queue-split gathers of the tile's lo/hi chunk runs."""
                nlo, nhi = NLO[ti], NHI[ti]
                lo_w, hi_w = WSTART[ti]
                Gv = G[:].rearrange("p (c e) -> p c e", e=tb)
                for side, n0, wb, c0, t0, t1 in (
                        (0, nlo, lo_w, 0, 0, LO),
                        (1, nhi, hi_w, nlo, HIOFF, NPAD)):
                    a = 0
                    while a < n0:
                        b = min(a + SUBCH, n0)
                        nb = b - a
                        nc.gpsimd.dma_gather(
                            Gv[:, c0 + a:c0 + b, :],
                            full[t0:t1, :], idx_sb[:, wb + 8 * a:wb + 8 * b],
                            128 * nb, 128 * nb, tb,
                            queue_num=qrot[0] % 4, single_packet=True)
                        qrot[0] += 1
                        a = b

            def gat1_tile(ti, psr, s):
                """Gather + attention + aggregation for one dst tile; the
                aggregation accumulates feature-major into psr[:, s*P:(s+1)*P]
                on top of the res1 projection already there."""
                nlo, nhi = NLO[ti], NHI[ti]
                ncc = nlo + nhi
                G = gp.tile([P, ncc * TB1], BF16, tag="G1")
                split_gather(G, ti, TB1, full1)
                G3 = G[:].rearrange("p (c e) -> p c e", e=TB1)
                Gf = G[:].bitcast(F32).rearrange("p (c f) -> p c f", f=P)
                # e = a_src + a_dst  (head-major [p, 4, ncc])
                ebuf = wk.tile([P, ncc * 4], F32, tag="ebuf")
                eb_h = ebuf[:].rearrange("p (f c) -> p f c", f=4)
                nc.vector.tensor_tensor(
                    out=eb_h, in0=Gf[:, :, 64:68].rearrange("p c f -> p f c"),
                    in1=adst_nm[:, ti * 4:(ti + 1) * 4].to_broadcast([P, 4, ncc]),
                    op=ALU.add)
                etmp = wk.tile([P, ncc * 4], F32, tag="etmp")
                nc.vector.tensor_scalar_mul(etmp[:], ebuf[:], SLOPE)
                nc.vector.tensor_tensor(out=ebuf[:], in0=ebuf[:], in1=etmp[:],
                                        op=ALU.max)
                nc.scalar.activation(ebuf[:], ebuf[:], AF.Exp)
                # mask (c-major view of head-major buffer)
                eb_c = ebuf[:].rearrange("p (f c) -> p c f", c=ncc)
                msl = mask_sb[:, CSTART[ti]:CSTART[ti] + ncc]
                nc.vector.tensor_tensor(out=eb_c, in0=eb_c,
                                        in1=msl.to_broadcast([P, ncc, 4]),
                                        op=ALU.mult)
                # alpha = e / sum_c e   (normalize before aggregation)
                dsum = wk.tile([P, 4], F32, tag="dsum")
                nc.vector.tensor_reduce(dsum[:], eb_h, AX.X, ALU.add)
                nc.vector.tensor_scalar_max(dsum[:], dsum[:], 1e-16)
                rec = wk.tile([P, 4], F32, tag="rec")
                nc.vector.reciprocal(rec[:], dsum[:])
                nc.vector.tensor_tensor(
                    out=eb_h, in0=eb_h,
                    in1=rec[:].to_broadcast([P, 4, ncc]),
                    op=ALU.mult)
                # G *= alpha (in place, bf16)
                g4 = G3[:, :, 0:P].rearrange("p c (f j) -> p c f j", j=HID)
                nc.vector.tensor_tensor(
                    out=g4, in0=g4,
                    in1=eb_c.to_broadcast([P, ncc, 4, HID]),
                    op=ALU.mult)
                # aggregate: psr[:, s*P:(s+1)*P][feat, slot] += sum_c G[slot, c, feat]
                for c in range(ncc):
                    nc.tensor.matmul(out=psr[:, s * P:(s + 1) * P],
                                     lhsT=G3[:, c, 0:P], rhs=identb_sb[:],
                                     start=False, stop=(c == ncc - 1))

            # ---- phase C/D interleaved per 512-node group
            for gi, gn in groups:
                GW = gn * P
                g0 = gi * 4 * P
                # res1 projection (feature-major) into psr
                psr = pb.tile([P, GW], F32, tag="pbig")
                for k in range(2):
                    nc.tensor.matmul(out=psr[:], lhsT=res1w_sb[:, k * P:(k + 1) * P],
                                     rhs=xe_res[:, k * NPC + g0:k * NPC + g0 + GW],
                                     start=(k == 0), stop=False)
                for s in range(gn):
                    gat1_tile(gi * 4 + s, psr, s)
                # h2 = relu(g1 + res1 + b)
                h2t = wk.tile([P, GW], BF16, tag="h2t")
                nc.scalar.activation(h2t[:], psr[:], AF.Relu, bias=b12_sb[:, 0:1])
                # z/b_src/b_dst projections + res2 (all feature-major)
                psz = pz.tile([42, GW], F32, tag="pzo")
                nc.tensor.matmul(out=psz[:], lhsT=l2_sb[:], rhs=h2t[:],
                                 start=True, stop=True)
                z_sd = wk.tile([42, GW], BF16, tag="zsd")
                nc.vector.tensor_copy(z_sd[:], psz[:])
                pso = pz.tile([OUT, GW], F32, tag="pzo")
                nc.tensor.matmul(out=pso[:], lhsT=r2w_sb[:], rhs=h2t[:],
                                 start=True, stop=True)
                nc.vector.tensor_copy(res2F[:, g0:g0 + GW], pso[:])
                for s in range(gn):
                    ti = gi * 4 + s
                    ptrz = ptp.tile([P, 42], BF16, tag="ptr")
                    nc.tensor.transpose(ptrz[:], z_sd[:, s * P:(s + 1) * P],
                                        identb_sb[0:42, 0:42])
                    tb2 = tbp.tile([P, TB2], BF16, tag="tb2")
                    nc.vector.tensor_copy(tb2[:, 0:41], ptrz[:, 0:41])
                    nc.vector.tensor_copy(bdst_nm[:, ti:ti + 1], ptrz[:, 41:42])
                    nc.sync.dma_start(slice2[ti * P:(ti + 1) * P, :], tb2[:])

            # ---- AllGather table2
            nc.gpsimd.collective_compute(
                "AllGather", ALU.bypass, replica_groups=[list(range(NCORE))],
                ins=[slice2[:]], outs=[full2[:]])

            # ---- phase E: GAT2 aggregation
            for ti in range(NTILE):
                nlo, nhi = NLO[ti], NHI[ti]
                ncc = nlo + nhi
                G2 = gp2.tile([P, ncc * TB2], BF16, tag="G2")
                split_gather(G2, ti, TB2, full2)
                G23 = G2[:].rearrange("p (c e) -> p c e", e=TB2)
                e2 = wk.tile([P, ncc], F32, tag="e2")
                nc.vector.tensor_tensor(
                    out=e2[:], in0=G23[:, :, 40:41].rearrange("p c f -> p (c f)"),
                    in1=bdst_nm[:, ti:ti + 1].to_broadcast([P, ncc]), op=ALU.add)
                e2tmp = wk.tile([P, ncc], F32, tag="e2tmp")
                nc.vector.tensor_scalar_mul(e2tmp[:], e2[:], SLOPE)
                nc.vector.tensor_tensor(out=e2[:], in0=e2[:], in1=e2tmp[:],
                                        op=ALU.max)
                nc.scalar.activation(e2[:], e2[:], AF.Exp)
                msl = mask_sb[:, CSTART[ti]:CSTART[ti] + ncc]
                nc.vector.tensor_tensor(out=e2[:], in0=e2[:], in1=msl, op=ALU.mult)
                dsum2 = wk.tile([P, 1], F32, tag="dsum2")
                nc.vector.tensor_reduce(dsum2[:], e2[:], AX.X, ALU.add)
                nc.vector.tensor_scalar_max(dsum2[:], dsum2[:], 1e-16)
                rec2 = wk.tile([P, 1], F32, tag="rec2")
                nc.vector.reciprocal(rec2[:], dsum2[:])
                nc.vector.tensor_tensor(out=e2[:], in0=e2[:],
                                        in1=rec2[:].to_broadcast([P, ncc]),
                                        op=ALU.mult)
                nc.vector.tensor_tensor(
                    out=G23[:, :, 0:OUT], in0=G23[:, :, 0:OUT],
                    in1=e2[:].to_broadcast([P, ncc, OUT]),
                    op=ALU.mult)
                po = ptp.tile([OUT, P], F32, tag="ptr")
                for c in range(ncc):
                    nc.tensor.matmul(out=po[:], lhsT=G23[:, c, 0:OUT],
                                     rhs=identb_sb[:],
                                     start=(c == 0), stop=(c == ncc - 1))
                ot = wk.tile([OUT, P], F32, tag="ot")
                nc.vector.scalar_tensor_tensor(
                    out=ot[:], in0=po[:], scalar=b2c_sb[0:OUT, 0:1],
                    in1=res2F[:, ti * P:(ti + 1) * P],
                    op0=ALU.add, op1=ALU.add)
                nc.sync.dma_start(out_d[:, ti * P:(ti + 1) * P], ot[:])

    nc.finalize()
    return nc


def kernel(**inputs):
    meta, in_maps, old_ids = _prepare(inputs)
    key = meta["key"]
    if key not in _cache:
        _cache[key] = _build(meta)
    nc = _cache[key]
    res = bass_utils.run_bass_kernel_spmd(nc, in_maps, core_ids=list(range(NCORE)))
    outp = np.zeros((NPAD, OUT), np.float32)
    for c in range(NCORE):
        outp[old_ids[c]] = res.results[c]["out"].T
    return outp[:N]


# revision 11
# speedup vs baseline: 1.0240x; 1.0240x over previous
"""GAT with autoencoder + residuals on 8 trn2 NeuronCores (Bass/Tile).

Strategy: nodes are renumbered by in-degree (desc) and dealt round-robin to the
8 cores; each dst-node owns one SBUF partition of its 128-node tile, and its
incoming edges occupy chunk columns of that partition. Gathers of source
features use dma_gather (int16 indices -> lo/hi split tables), split across
the 4 SWDGE queues to avoid ring backpressure. Attention weights are
normalized (alpha = e/sum e) BEFORE aggregation; aggregation runs
data-stationary on the tensor engine (lhsT = gathered chunk, rhs = identity),
accumulating feature-major directly on top of the residual-projection PSUM.
All dense math in bf16. Small weights are replicated; two AllGathers publish
the per-core projection tables between phases.
"""
import hashlib
import numpy as np
import ml_dtypes

import concourse.bacc as bacc
import concourse.mybir as mybir
import concourse.tile as tile
from concourse import bass_utils

# model sizes (fixed by the problem)
N = 50000
IN = 512
ENC = 256
HID = 32
HEADS = 4
OUT = 40
SLOPE = 0.2

NCORE = 8
P = 128
NTILE = 49
NPC = NTILE * P          # 6272 nodes per core
NPAD = NCORE * NPC       # 50176
LO = 32768               # lo table rows [0, LO)
HIOFF = NPAD - 32768     # hi table rows [HIOFF, NPAD)
# chunk-major table layout: tables are built in CHUNKS of tile ranges so the
# AllGather of chunk k can launch as soon as its producer groups finish.
CH_T = [0, 16, 28, 40, 49]           # tile boundaries of the 4 chunks
CH_R0 = [t * P for t in CH_T]        # per-core row boundaries
CH_CR = [CH_R0[i + 1] - CH_R0[i] for i in range(4)]   # rows per core per chunk
CH_CB = [0]
for i in range(4):
    CH_CB.append(CH_CB[-1] + NCORE * CH_CR[i])        # table base of chunk

F32 = mybir.dt.float32
BF16 = mybir.dt.bfloat16
I16 = mybir.dt.int16
AF = mybir.ActivationFunctionType
ALU = mybir.AluOpType
AX = mybir.AxisListType

TB1 = 256   # bf16 columns per table1 row: [h(128 bf16) | a_src(4 f32) | pad]
TB2 = 128   # bf16 columns per table2 row: [z(40 bf16) | b_src(1 bf16) | pad]
SUBCH = 6   # max chunks per sub-gather

_cache = {}


def _wrap_idx(blk):
    """[128, NB] slot-major block -> dma_gather idx layout [128, 8*NB] int16."""
    nb = blk.shape[1]
    flat = blk.T.reshape(-1)                 # j = c*128 + p
    w = flat.reshape(-1, 16).T               # [16, 8*NB]
    return np.tile(w, (8, 1)).astype(np.int16)


def _prepare(inputs):
    x = np.asarray(inputs["x"], np.float32)
    ei = np.asarray(inputs["edge_index"]).astype(np.int64)
    src = np.concatenate([ei[0], np.arange(N, dtype=np.int64)])
    dst = np.concatenate([ei[1], np.arange(N, dtype=np.int64)])

    deg = np.bincount(dst, minlength=NPAD)
    order = np.argsort(-deg, kind="stable")
    rank = np.empty(NPAD, np.int64)
    rank[order] = np.arange(NPAD)
    core_of = rank % NCORE
    pos_of = rank // NCORE
    tidx_of = core_of * NPC + pos_of         # table row of each (old) node

    er = rank[dst]
    est = tidx_of[src]
    lo_ex = est < HIOFF
    hi_ex = est >= LO
    key2 = np.where(lo_ex, 0, np.where(hi_ex, 2, 1))

    nlo = np.bincount(er[lo_ex], minlength=NPAD)
    nhi = np.bincount(er[hi_ex], minlength=NPAD)
    degr = deg[order]

    NLO = np.zeros(NTILE, np.int64)
    NHI = np.zeros(NTILE, np.int64)
    l_of = np.zeros(NPAD, np.int64)
    B = NCORE * P
    for t in range(NTILE):
        blk = slice(t * B, (t + 1) * B)
        NLO[t] = nlo[blk].max()
        l = np.minimum(degr[blk] - nhi[blk], NLO[t])
        l_of[blk] = l
        NHI[t] = max(nhi[blk].max(), (degr[blk] - l).max(), 0)
    NC = NLO + NHI
    CSTART = np.zeros(NTILE, np.int64)
    CSTART[1:] = np.cumsum(NC)[:-1]
    CTOT = int(NC.sum())

    # per-edge slot assignment
    eord = np.lexsort((key2, er))
    er_s = er[eord]
    est_s = est[eord]
    boundaries = np.flatnonzero(np.r_[True, er_s[1:] != er_s[:-1]])
    counts = np.diff(np.r_[boundaries, er_s.size])
    j = np.arange(er_s.size) - np.repeat(boundaries, counts)
    lcap = l_of[er_s]
    side_lo = j < lcap
    et_s = (er_s // NCORE) // P
    epart_s = (er_s // NCORE) % P
    ecore_s = er_s % NCORE
    col = np.where(side_lo, j, NLO[et_s] + (j - lcap))
    val = np.where(side_lo, est_s, est_s - HIOFF).astype(np.int16)

    sval = np.zeros((NCORE, P, CTOT), np.int16)
    mask = np.zeros((NCORE, P, CTOT), np.float32)
    colg = CSTART[et_s] + col
    sval[ecore_s, epart_s, colg] = val
    mask[ecore_s, epart_s, colg] = 1.0

    # wrapped idx blocks per (tile, side) concatenated; WSTART per gather
    WSTART = []
    w_off = 0
    for t in range(NTILE):
        WSTART.append((w_off, w_off + 8 * int(NLO[t])))
        w_off += 8 * int(NLO[t]) + 8 * int(NHI[t])
    WTOT = w_off
    idx_all = np.zeros((NCORE, P, WTOT), np.int16)
    for c in range(NCORE):
        for t in range(NTILE):
            cs = CSTART[t]
            lo_w, hi_w = WSTART[t]
            if NLO[t]:
                idx_all[c][:, lo_w:lo_w + 8 * int(NLO[t])] = _wrap_idx(
                    sval[c][:, cs:cs + int(NLO[t])])
            if NHI[t]:
                idx_all[c][:, hi_w:hi_w + 8 * int(NHI[t])] = _wrap_idx(
                    sval[c][:, cs + int(NLO[t]):cs + int(NC[t])])

    # per-core x (transposed, feature-major, bf16)
    xp = np.zeros((NPAD, IN), np.float32)
    xp[:N] = x
    old_ids = np.empty((NCORE, NPC), np.int64)
    x_t = np.empty((NCORE, IN, NPC), ml_dtypes.bfloat16)
    for c in range(NCORE):
        ids = order[c + NCORE * np.arange(NPC)]
        old_ids[c] = ids
        x_t[c] = xp[ids].T.astype(ml_dtypes.bfloat16)

    # replicated derived weights
    w = lambda k: np.asarray(inputs[k], np.float32)
    bf = lambda a: np.ascontiguousarray(a).astype(ml_dtypes.bfloat16)
    a1sd = np.zeros((P, 8), np.float32)
    for h in range(HEADS):
        a1sd[h * HID:(h + 1) * HID, h] = w("gat1_att_src")[h]
        a1sd[h * HID:(h + 1) * HID, 4 + h] = w("gat1_att_dst")[h]
    vs2 = w("gat2_w") @ w("gat2_att_src")[0]
    vd2 = w("gat2_w") @ w("gat2_att_dst")[0]
    lhsT2 = np.concatenate([w("gat2_w"), vs2[:, None], vd2[:, None]], 1)  # [128,42]
    bias12 = (w("gat1_b") + w("res1_b")).reshape(P, 1)
    b2c = np.zeros((P, 1), np.float32)
    b2c[:OUT, 0] = w("gat2_b") + w("res2_b")

    consts = {
        "identb": np.eye(P, dtype=np.float32).astype(ml_dtypes.bfloat16),
        "ident": np.eye(P, dtype=np.float32),
        "w1": bf(w("ae_w1")), "b1p": w("ae_b1").reshape(2, P).T.copy(),
        "w2": bf(w("ae_w2")), "b2p": w("ae_b2").reshape(2, P).T.copy(),
        "g1w": bf(w("gat1_w")), "a1sd": bf(a1sd),
        "res1w": bf(w("res1_w")), "b12p": bias12,
        "l2p": bf(lhsT2), "r2w": bf(w("res2_w")), "b2cp": b2c,
    }
    meta = {
        "NLO": NLO.tolist(), "NHI": NHI.tolist(),
        "CSTART": CSTART.tolist(), "CTOT": CTOT,
        "WSTART": WSTART, "WTOT": WTOT,
        "key": "v2:" + hashlib.sha1(ei.tobytes()).hexdigest(),
    }
    in_maps = []
    for c in range(NCORE):
        m = {"x_t": x_t[c], "idx_all": idx_all[c], "mask_all": mask[c]}
        m.update(consts)
        in_maps.append(m)
    return meta, in_maps, old_ids


def _build(meta):
    NLO, NHI = meta["NLO"], meta["NHI"]
    CSTART, CTOT = meta["CSTART"], meta["CTOT"]
    WSTART, WTOT = meta["WSTART"], meta["WTOT"]

    nc = bacc.Bacc("TRN2", target_bir_lowering=False, debug=False,
                   num_devices=NCORE, num_swdge_queues=4)
    # I/O
    x_t = nc.dram_tensor("x_t", [IN, NPC], BF16, kind="ExternalInput")
    idx_all = nc.dram_tensor("idx_all", [P, WTOT], I16, kind="ExternalInput")
    mask_all = nc.dram_tensor("mask_all", [P, CTOT], F32, kind="ExternalInput")
    identb = nc.dram_tensor("identb", [P, P], BF16, kind="ExternalInput")
    ident = nc.dram_tensor("ident", [P, P], F32, kind="ExternalInput")
    w1 = nc.dram_tensor("w1", [IN, ENC], BF16, kind="ExternalInput")
    b1p = nc.dram_tensor("b1p", [P, 2], F32, kind="ExternalInput")
    w2 = nc.dram_tensor("w2", [ENC, ENC], BF16, kind="ExternalInput")
    b2p = nc.dram_tensor("b2p", [P, 2], F32, kind="ExternalInput")
    g1w = nc.dram_tensor("g1w", [ENC, P], BF16, kind="ExternalInput")
    a1sd = nc.dram_tensor("a1sd", [P, 8], BF16, kind="ExternalInput")
    res1w = nc.dram_tensor("res1w", [ENC, P], BF16, kind="ExternalInput")
    b12p = nc.dram_tensor("b12p", [P, 1], F32, kind="ExternalInput")
    l2p = nc.dram_tensor("l2p", [P, 42], BF16, kind="ExternalInput")
    r2w = nc.dram_tensor("r2w", [P, OUT], BF16, kind="ExternalInput")
    b2cp = nc.dram_tensor("b2cp", [P, 1], F32, kind="ExternalInput")
    out_d = nc.dram_tensor("out", [OUT, NPC], F32, kind="ExternalOutput")

    groups = [(gi, min(4, NTILE - gi * 4)) for gi in range((NTILE + 3) // 4)]

    with tile.TileContext(nc) as tc:
        with (
            tc.tile_pool(name="const", bufs=1) as cp,
            tc.tile_pool(name="pers", bufs=1) as pp,
            tc.tile_pool(name="dram", bufs=1, space="DRAM") as dp,
            tc.tile_pool(name="xk", bufs=2) as xkp,
            tc.tile_pool(name="wk", bufs=2) as wk,
            tc.tile_pool(name="tb", bufs=3) as tbp,
            tc.tile_pool(name="gg", bufs=2) as gp,
            tc.tile_pool(name="gg2", bufs=3) as gp2,
            tc.tile_pool(name="pbig", bufs=2, space="PSUM") as pb,
            tc.tile_pool(name="pzo", bufs=2, space="PSUM") as pz,
            tc.tile_pool(name="ptr", bufs=4, space="PSUM") as ptp,
        ):
            # ---- load constants
            identb_sb = cp.tile([P, P], BF16)
            nc.sync.dma_start(identb_sb[:], identb[:])
            ident_sb = cp.tile([P, P], F32)
            nc.sync.dma_start(ident_sb[:], ident[:])
            w1_sb = cp.tile([P, 4 * ENC], BF16)
            nc.sync.dma_start(
                w1_sb[:].rearrange("p (k m) -> p k m", m=ENC),
                w1[:].rearrange("(k p) m -> p k m", p=P))
            w2_sb = cp.tile([P, 2 * ENC], BF16)
            nc.sync.dma_start(
                w2_sb[:].rearrange("p (k m) -> p k m", m=ENC),
                w2[:].rearrange("(k p) m -> p k m", p=P))
            g1w_sb = cp.tile([P, 2 * P], BF16)
            nc.sync.dma_start(
                g1w_sb[:].rearrange("p (k m) -> p k m", m=P),
                g1w[:].rearrange("(k p) m -> p k m", p=P))
            res1w_sb = cp.tile([P, 2 * P], BF16)
            nc.sync.dma_start(
                res1w_sb[:].rearrange("p (k m) -> p k m", m=P),
                res1w[:].rearrange("(k p) m -> p k m", p=P))
            b1_sb = cp.tile([P, 2], F32)
            nc.sync.dma_start(b1_sb[:], b1p[:])
            b2_sb = cp.tile([P, 2], F32)
            nc.sync.dma_start(b2_sb[:], b2p[:])
            a1sd_sb = cp.tile([P, 8], BF16)
            nc.sync.dma_start(a1sd_sb[:], a1sd[:])
            b12_sb = cp.tile([P, 1], F32)
            nc.sync.dma_start(b12_sb[:], b12p[:])
            l2_sb = cp.tile([P, 42], BF16)
            nc.sync.dma_start(l2_sb[:], l2p[:])
            r2w_sb = cp.tile([P, OUT], BF16)
            nc.sync.dma_start(r2w_sb[:], r2w[:])
            b2c_sb = cp.tile([P, 1], F32)
            nc.sync.dma_start(b2c_sb[:], b2cp[:])
            idx_sb = pp.tile([P, WTOT], I16)
            nc.sync.dma_start(idx_sb[:], idx_all[:])
            mask_sb = pp.tile([P, CTOT], F32)
            nc.sync.dma_start(mask_sb[:], mask_all[:])

            adst_nm = pp.tile([P, NTILE * 4], F32)
            bdst_nm = pp.tile([P, NTILE], F32)
            xe_res = pp.tile([P, 2 * NPC], BF16)       # encoder output, resident
            res2F = pp.tile([OUT, NPC], F32)           # res2 branch, feature-major

            # internal DRAM
            slice1 = dp.tile([NPC, TB1], BF16)
            full1 = dp.tile([NPAD, TB1], BF16, addr_space="Shared")
            slice2 = dp.tile([NPC, TB2], BF16)
            full2 = dp.tile([NPAD, TB2], BF16, addr_space="Shared")

            # ---- phase A+B: autoencoder, GAT1 projections, table1 rows
            for gi, gn in groups:
                GW = gn * P
                g0 = gi * 4 * P
                xks = []
                for k in range(4):
                    xk = xkp.tile([P, GW], BF16, tag=f"xk{k}")
                    nc.sync.dma_start(xk[:], x_t[k * P:(k + 1) * P, g0:g0 + GW])
                    xks.append(xk)
                z1s = []
                for m in range(2):
                    ps1 = pb.tile([P, GW], F32, tag="pbig")
                    for k in range(4):
                        nc.tensor.matmul(
                            out=ps1[:], lhsT=w1_sb[:, k * ENC + m * P:k * ENC + (m + 1) * P],
                            rhs=xks[k][:], start=(k == 0), stop=(k == 3))
                    z1 = wk.tile([P, GW], BF16, tag=f"z1{m}")
                    nc.scalar.activation(z1[:], ps1[:], AF.Relu, bias=b1_sb[:, m:m + 1])
                    z1s.append(z1)
                for m in range(2):
                    ps2 = pb.tile([P, GW], F32, tag="pbig")
                    for k in range(2):
                        nc.tensor.matmul(
                            out=ps2[:], lhsT=w2_sb[:, k * ENC + m * P:k * ENC + (m + 1) * P],
                            rhs=z1s[k][:], start=(k == 0), stop=(k == 1))
                    nc.scalar.activation(xe_res[:, m * NPC + g0:m * NPC + g0 + GW],
                                         ps2[:], AF.Relu, bias=b2_sb[:, m:m + 1])
                # h = xe @ gat1_w ; a_src/a_dst
                psh = pb.tile([P, GW], F32, tag="pbig")
                for k in range(2):
                    nc.tensor.matmul(out=psh[:], lhsT=g1w_sb[:, k * P:(k + 1) * P],
                                     rhs=xe_res[:, k * NPC + g0:k * NPC + g0 + GW],
                                     start=(k == 0), stop=(k == 1))
                h_sb = wk.tile([P, GW], BF16, tag="hsb")
                nc.vector.tensor_copy(h_sb[:], psh[:])
                psa = pz.tile([8, GW], F32, tag="pzo")
                nc.tensor.matmul(out=psa[:], lhsT=a1sd_sb[:], rhs=h_sb[:],
                                 start=True, stop=True)
                a_sd = wk.tile([8, GW], F32, tag="asd")
                nc.vector.tensor_copy(a_sd[:], psa[:])
                for s in range(gn):
                    ti = gi * 4 + s
                    ptr1 = ptp.tile([P, P], BF16, tag="ptr")
                    nc.tensor.transpose(ptr1[:], h_sb[:, s * P:(s + 1) * P], identb_sb[:])
                    tb1 = tbp.tile([P, TB1], BF16, tag="tb1")
                    nc.vector.tensor_copy(tb1[:, 0:P], ptr1[:])
                    ptr2 = ptp.tile([P, 8], F32, tag="ptr")
                    nc.tensor.transpose(ptr2[:], a_sd[:, s * P:(s + 1) * P], ident_sb[0:8, 0:8])
                    f32v = tb1[:].bitcast(F32)
                    nc.vector.tensor_copy(f32v[:, 64:68], ptr2[:, 0:4])
                    nc.vector.tensor_copy(adst_nm[:, ti * 4:(ti + 1) * 4], ptr2[:, 4:8])
                    nc.sync.dma_start(slice1[ti * P:(ti + 1) * P, :], tb1[:])

            # ---- AllGather table1
            nc.gpsimd.collective_compute(
                "AllGather", ALU.bypass, replica_groups=[list(range(NCORE))],
                ins=[slice1[:]], outs=[full1[:]])

            qrot = [0]

            def split_gather(G, ti, tb, full):
                """4-queue-split gathers of the tile's lo/hi chunk runs."""
                nlo, nhi = NLO[ti], NHI[ti]
                lo_w, hi_w = WSTART[ti]
                Gv = G[:].rearrange("p (c e) -> p c e", e=tb)
                for side, n0, wb, c0, t0, t1 in (
                        (0, nlo, lo_w, 0, 0, LO),
                        (1, nhi, hi_w, nlo, HIOFF, NPAD)):
                    a = 0
                    while a < n0:
                        b = min(a + SUBCH, n0)
                        nb = b - a
                        nc.gpsimd.dma_gather(
                            Gv[:, c0 + a:c0 + b, :],
                            full[t0:t1, :], idx_sb[:, wb + 8 * a:wb + 8 * b],
                            128 * nb, 128 * nb, tb,
                            queue_num=qrot[0] % 4, single_packet=True)
                        qrot[0] += 1
                        a = b

            def gat1_tile(ti, psr, s):
                """Gather + attention + aggregation for one dst tile; the
                aggregation accumulates feature-major into psr[:, s*P:(s+1)*P]
                on top of the res1 projection already there."""
                nlo, nhi = NLO[ti], NHI[ti]
                ncc = nlo + nhi
                G = gp.tile([P, ncc * TB1], BF16, tag="G1")
                split_gather(G, ti, TB1, full1)
                G3 = G[:].rearrange("p (c e) -> p c e", e=TB1)
                Gf = G[:].bitcast(F32).rearrange("p (c f) -> p c f", f=P)
                # e = a_src + a_dst  (head-major [p, 4, ncc])
                ebuf = wk.tile([P, ncc * 4], F32, tag="ebuf")
                eb_h = ebuf[:].rearrange("p (f c) -> p f c", f=4)
                nc.vector.tensor_tensor(
                    out=eb_h, in0=Gf[:, :, 64:68].rearrange("p c f -> p f c"),
                    in1=adst_nm[:, ti * 4:(ti + 1) * 4].to_broadcast([P, 4, ncc]),
                    op=ALU.add)
                etmp = wk.tile([P, ncc * 4], F32, tag="etmp")
                nc.vector.tensor_scalar_mul(etmp[:], ebuf[:], SLOPE)
                nc.vector.tensor_tensor(out=ebuf[:], in0=ebuf[:], in1=etmp[:],
                                        op=ALU.max)
                nc.scalar.activation(ebuf[:], ebuf[:], AF.Exp)
                # mask (c-major view of head-major buffer)
                eb_c = ebuf[:].rearrange("p (f c) -> p c f", c=ncc)
                msl = mask_sb[:, CSTART[ti]:CSTART[ti] + ncc]
                nc.vector.tensor_tensor(out=eb_c, in0=eb_c,
                                        in1=msl.to_broadcast([P, ncc, 4]),
                                        op=ALU.mult)
                # alpha = e / sum_c e   (normalize before aggregation)
                dsum = wk.tile([P, 4], F32, tag="dsum")
                nc.vector.tensor_reduce(dsum[:], eb_h, AX.X, ALU.add)
                nc.vector.tensor_scalar_max(dsum[:], dsum[:], 1e-16)
                rec = wk.tile([P, 4], F32, tag="rec")
                nc.vector.reciprocal(rec[:], dsum[:])
                nc.vector.tensor_tensor(
                    out=eb_h, in0=eb_h,
                    in1=rec[:].to_broadcast([P, 4, ncc]),
                    op=ALU.mult)
                # G *= alpha (in place, bf16)
                g4 = G3[:, :, 0:P].rearrange("p c (f j) -> p c f j", j=HID)
                nc.vector.tensor_tensor(
                    out=g4, in0=g4,
                    in1=eb_c.to_broadcast([P, ncc, 4, HID]),
                    op=ALU.mult)
                # aggregate: psr[:, s*P:(s+1)*P][feat, slot] += sum_c G[slot, c, feat]
                for c in range(ncc):
                    nc.tensor.matmul(out=psr[:, s * P:(s + 1) * P],
                                     lhsT=G3[:, c, 0:P], rhs=identb_sb[:],
                                     start=False, stop=(c == ncc - 1))

            # ---- phase C/D interleaved per 512-node group
            for gi, gn in groups:
                GW = gn * P
                g0 = gi * 4 * P
                # res1 projection (feature-major) into psr
                psr = pb.tile([P, GW], F32, tag="pbig")
                for k in range(2):
                    nc.tensor.matmul(out=psr[:], lhsT=res1w_sb[:, k * P:(k + 1) * P],
                                     rhs=xe_res[:, k * NPC + g0:k * NPC + g0 + GW],
                                     start=(k == 0), stop=False)
                for s in range(gn):
                    gat1_tile(gi * 4 + s, psr, s)
                # h2 = relu(g1 + res1 + b)
                h2t = wk.tile([P, GW], BF16, tag="h2t")
                nc.scalar.activation(h2t[:], psr[:], AF.Relu, bias=b12_sb[:, 0:1])
                # z/b_src/b_dst projections + res2 (all feature-major)
                psz = pz.tile([42, GW], F32, tag="pzo")
                nc.tensor.matmul(out=psz[:], lhsT=l2_sb[:], rhs=h2t[:],
                                 start=True, stop=True)
                z_sd = wk.tile([42, GW], BF16, tag="zsd")
                nc.vector.tensor_copy(z_sd[:], psz[:])
                pso = pz.tile([OUT, GW], F32, tag="pzo")
                nc.tensor.matmul(out=pso[:], lhsT=r2w_sb[:], rhs=h2t[:],
                                 start=True, stop=True)
                nc.vector.tensor_copy(res2F[:, g0:g0 + GW], pso[:])
                for s in range(gn):
                    ti = gi * 4 + s
                    ptrz = ptp.tile([P, 42], BF16, tag="ptr")
                    nc.tensor.transpose(ptrz[:], z_sd[:, s * P:(s + 1) * P],
                                        identb_sb[0:42, 0:42])
                    tb2 = tbp.tile([P, TB2], BF16, tag="tb2")
                    nc.vector.tensor_copy(tb2[:, 0:41], ptrz[:, 0:41])
                    nc.vector.tensor_copy(bdst_nm[:, ti:ti + 1], ptrz[:, 41:42])
                    nc.sync.dma_start(slice2[ti * P:(ti + 1) * P, :], tb2[:])

            # ---- AllGather table2
            nc.gpsimd.collective_compute(
                "AllGather", ALU.bypass, replica_groups=[list(range(NCORE))],
                ins=[slice2[:]], outs=[full2[:]])

            # ---- phase E: GAT2 aggregation
            for ti in range(NTILE):
                nlo, nhi = NLO[ti], NHI[ti]
                ncc = nlo + nhi
                G2 = gp2.tile([P, ncc * TB2], BF16, tag="G2")
                split_gather(G2, ti, TB2, full2)
                G23 = G2[:].rearrange("p (c e) -> p c e", e=TB2)
                e2 = wk.tile([P, ncc], F32, tag="e2")
                nc.vector.tensor_tensor(
                    out=e2[:], in0=G23[:, :, 40:41].rearrange("p c f -> p (c f)"),
                    in1=bdst_nm[:, ti:ti + 1].to_broadcast([P, ncc]), op=ALU.add)
                e2tmp = wk.tile([P, ncc], F32, tag="e2tmp")
                nc.vector.tensor_scalar_mul(e2tmp[:], e2[:], SLOPE)
                nc.vector.tensor_tensor(out=e2[:], in0=e2[:], in1=e2tmp[:],
                                        op=ALU.max)
                nc.scalar.activation(e2[:], e2[:], AF.Exp)
                msl = mask_sb[:, CSTART[ti]:CSTART[ti] + ncc]
                nc.vector.tensor_tensor(out=e2[:], in0=e2[:], in1=msl, op=ALU.mult)
                dsum2 = wk.tile([P, 1], F32, tag="dsum2")
                nc.vector.tensor_reduce(dsum2[:], e2[:], AX.X, ALU.add)
                nc.vector.tensor_scalar_max(dsum2[:], dsum2[:], 1e-16)
                rec2 = wk.tile([P, 1], F32, tag="rec2")
                nc.vector.reciprocal(rec2[:], dsum2[:])
                nc.vector.tensor_tensor(out=e2[:], in0=e2[:],
                                        in1=rec2[:].to_broadcast([P, ncc]),
                                        op=ALU.mult)
                nc.vector.tensor_tensor(
                    out=G23[:, :, 0:OUT], in0=G23[:, :, 0:OUT],
                    in1=e2[:].to_broadcast([P, ncc, OUT]),
                    op=ALU.mult)
                po = ptp.tile([OUT, P], F32, tag="ptr")
                for c in range(ncc):
                    nc.tensor.matmul(out=po[:], lhsT=G23[:, c, 0:OUT],
                                     rhs=identb_sb[:],
                                     start=(c == 0), stop=(c == ncc - 1))
                ot = wk.tile([OUT, P], F32, tag="ot")
                nc.vector.scalar_tensor_tensor(
                    out=ot[:], in0=po[:], scalar=b2c_sb[0:OUT, 0:1],
                    in1=res2F[:, ti * P:(ti + 1) * P],
                    op0=ALU.add, op1=ALU.add)
                nc.sync.dma_start(out_d[:, ti * P:(ti + 1) * P], ot[:])

    nc.finalize()
    return nc


def kernel(**inputs):
    meta, in_maps, old_ids = _prepare(inputs)
    key = meta["key"]
    if key not in _cache:
        _cache[key] = _build(meta)
    nc = _cache[key]
    res = bass_utils.run_bass_kernel_spmd(nc, in_maps, core_ids=list(range(NCORE)))
    outp = np.zeros((NPAD, OUT), np.float32)
    for c in range(NCORE):
        outp[old_ids[c]] = res.results[c]["out"].T
    return outp[:N]
